# revision 26
# baseline (speedup 1.0000x reference)
"""Trainium2 Bass kernel for a 2-layer GCN regressor (gnn_message_passing).

Computation (matches the reference):
  deg_out/deg_in -> norm = max(deg,1)^-0.5
  layer:  x = (h * norm_src) @ W ; agg = segment_sum(x[src], dst) ;
          h' = relu(agg * norm_dst + b)
  pool:   per-graph mean over nodes, then hg @ W3 + b3 -> [G, 1]

Sharding: nodes are partitioned contiguously across the 8 cores (dst
partitioning).  Each core projects its owned nodes, the projected features are
AllGathered so every core holds the full [N,128] bf16 table in HBM, then each
core aggregates its owned destination super-blocks (256 nodes).

The gather of source rows is the bottleneck (SWDGE descriptor generation on
the gpsimd Q7 cluster is ~8.5 ns/row regardless of instruction).  To minimise
per-instruction overhead the kernel uses dma_gather (mlp ucode library) with
1024 rows per instruction.  dma_gather indices are int16, so the table is
viewed as 50176 pair-rows of 512B (two nodes per row) and split into two
ranges (<32768 rows each).  Edges are packed into 128-slot tiles per
(dst-super-block, range); each gathered 512B slot holds the needed node in
its even or odd half.  A single is_equal against a 512-wide fp16 iota builds
a parity-aware one-hot S4 (col = parity*256 + dst_slot), and two matmuls
accumulate agg^T[128f, 256d] = X_e.T @ S4[:, :256] + X_o.T @ S4[:, 256:].

All heavy matmul traffic is bf16 (one-hot values exact); accumulation fp32.
"""

import numpy as np
import ml_dtypes

BF16 = ml_dtypes.bfloat16
NC = 8          # cores
GT = 8          # gather tiles (of 128 rows) per dma_gather instruction
RSPLIT = 32768  # pair-rows per int16-addressable range


# ----------------------------------------------------------------- host prep
def _prep(h, src, dst, graph_ids, num_graphs, W1, b1, W2, b2, W3, b3):
    h = np.asarray(h, dtype=np.float32)
    src = np.asarray(src, dtype=np.int64)
    dst = np.asarray(dst, dtype=np.int64)
    gid = np.asarray(graph_ids, dtype=np.int64)
    G = int(num_graphs)
    N, D = h.shape
    assert D == 128
    assert N % NC == 0
    NPC = N // NC               # real nodes per core
    B = (NPC + 127) // 128      # 128-node blocks per core
    NPAD = B * 128
    assert B % 2 == 0
    SBK = B                     # tile-grid granularity: one 128-node block
    assert G % 128 == 0
    GC = G // 128               # output column groups

    owner = dst // NPC                      # edge -> owning core
    d_loc = dst - owner * NPC               # local node id on owner
    sblk = d_loc // 128
    jsb = d_loc - sblk * 128                # slot within block
    # source half-table id / pair-row within half / parity
    HB2 = (NPAD // 2)                       # nodes per half per core
    s_owner = src // NPC
    s_pos = src - s_owner * NPC
    rng = (s_pos >= HB2).astype(np.int64)   # half 0 / 1
    s_u = s_owner * (HB2 // 2) + ((s_pos - rng * HB2) >> 1)
    s_par = (s_pos & 1).astype(np.int64)

    # per (core, sblock, range) edge counts -> shared tile grid
    counts = np.zeros((NC, SBK, 2), dtype=np.int64)
    np.add.at(counts, (owner, sblk, rng), 1)
    T_r = np.maximum(1, (counts.max(axis=0) + 127) // 128)  # [SBK, 2]
    TA_list = T_r[:, 0]
    TB_list = T_r[:, 1]
    TA_tot = int(TA_list.sum())
    TB_tot = int(TB_list.sum())
    T_tot = TA_tot + TB_tot
    offA = np.zeros(SBK, dtype=np.int64)
    offA[1:] = np.cumsum(TA_list)[:-1]
    offB = np.zeros(SBK, dtype=np.int64)
    offB[1:] = np.cumsum(TB_list)[:-1]

    # CSR row pointers (degree metadata) over real node ids
    src_sorted = np.sort(src)
    dst_sorted = np.sort(dst)
    rp_src = np.searchsorted(src_sorted, np.arange(N + 1)).astype(np.int32)
    rp_dst = np.searchsorted(dst_sorted, np.arange(N + 1)).astype(np.int32)

    per_core = []
    order_all = np.lexsort((s_u, rng, sblk, owner))
    e_owner = owner[order_all]
    e_sblk = sblk[order_all]
    e_rng = rng[order_all]
    e_j = jsb[order_all]
    e_u = s_u[order_all]
    e_par = s_par[order_all]
    core_starts = np.searchsorted(e_owner, np.arange(NC + 1))

    def wrap16(seq):
        """int16 wrapped layout [128, len/16]: pos i -> [i%16, i//16], x8."""
        n = len(seq)
        assert n % 16 == 0
        w = np.zeros((128, n // 16), dtype=np.int16)
        t = seq.reshape(-1, 16).T.astype(np.int16)
        for g in range(8):
            w[g * 16:(g + 1) * 16, :] = t
        return w

    for k in range(NC):
        lo, hi = core_starts[k], core_starts[k + 1]
        kb = e_sblk[lo:hi]
        kr = e_rng[lo:hi]
        kj = e_j[lo:hi]
        ku = e_u[lo:hi]
        kp = e_par[lo:hi]
        key = kb * 2 + kr
        bkt_starts = np.searchsorted(key, np.arange(2 * SBK + 1))

        idxA = np.zeros(TA_tot * 128, dtype=np.int64)
        idxB = np.zeros(TB_tot * 128, dtype=np.int64)
        dloc4 = np.full((128, T_tot), -1.0, dtype=np.float32)
        for b in range(SBK):
            for r, (idx_arr, off_r, base_col) in enumerate(
                    ((idxA, offA, 0), (idxB, offB, TA_tot))):
                s, e = bkt_starts[b * 2 + r], bkt_starts[b * 2 + r + 1]
                cnt = e - s
                if cnt == 0:
                    continue
                i = np.arange(cnt)
                t0 = off_r[b]
                slot = t0 * 128 + i          # sequence position in range r
                idx_arr[slot] = ku[s:e]
                col = base_col + t0 + i // 128
                row = i % 128
                dloc4[row, col] = (kp[s:e] * 128 + kj[s:e]).astype(np.float32)

        idxA16 = wrap16(idxA)
        idxB16 = wrap16(idxB)

        # graph one-hot over this core's graph window
        n0 = k * NPC
        kgid = gid[n0:n0 + NPC]
        g_base = int(kgid[0])
        span = int(kgid[-1]) - g_base + 1
        assert span <= 128, f"graph span {span} > 128 on core {k}"
        gone = np.zeros((128, NPAD), dtype=BF16)
        p_all = np.arange(NPC)
        gone[p_all % 128, (p_all // 128) * 128 + (kgid - g_base)] = BF16(1.0)
        gidx = (g_base + np.arange(128)).astype(np.int32)[:, None]

        # row-pointer tiles [128, B] (pad slots -> deg 0)
        node = n0 + np.arange(NPAD)
        valid = node < n0 + NPC
        nn = np.where(valid, node, n0)
        rsl = np.where(valid, rp_src[nn], 0).astype(np.int32).reshape(B, 128).T
        rsh = np.where(valid, rp_src[nn + 1], 0).astype(np.int32).reshape(B, 128).T
        rdl = np.where(valid, rp_dst[nn], 0).astype(np.int32).reshape(B, 128).T
        rdh = np.where(valid, rp_dst[nn + 1], 0).astype(np.int32).reshape(B, 128).T

        deg_out = (rp_src[n0 + 1:n0 + NPC + 1] - rp_src[n0:n0 + NPC])
        nsrc = 1.0 / np.sqrt(np.maximum(deg_out, 1.0))
        hT = np.zeros((128, NPAD), dtype=np.float32)
        hT[:, :NPC] = (h[n0:n0 + NPC] * nsrc[:, None]).T

        per_core.append(dict(
            hT=np.ascontiguousarray(hT),
            idxA16=np.ascontiguousarray(idxA16),
            idxB16=np.ascontiguousarray(idxB16),
            dloc4=np.ascontiguousarray(dloc4),
            gonehot=gone, gidx=gidx,
            rsl=np.ascontiguousarray(rsl), rsh=np.ascontiguousarray(rsh),
            rdl=np.ascontiguousarray(rdl), rdh=np.ascontiguousarray(rdh),
        ))

    iota4 = np.broadcast_to(np.tile(np.arange(256, dtype=np.float32), 8),
                            (128, 2048)).astype(BF16)
    ident = np.eye(128, dtype=np.float32)
    shared = dict(
        W1=np.asarray(W1, np.float32), W2=np.asarray(W2, np.float32),
        W3=np.asarray(W3, np.float32).reshape(128, 1),
        b1=np.asarray(b1, np.float32).reshape(128, 1),
        b2=np.asarray(b2, np.float32).reshape(128, 1),
        b3=np.broadcast_to(np.asarray(b3, np.float32).reshape(1, 1),
                           (128, 1)).astype(np.float32),
        iota4=np.ascontiguousarray(iota4),
        ident=np.ascontiguousarray(ident),
    )
    in_maps = [{**shared, **pc} for pc in per_core]
    cfg = dict(N=N, NPC=NPC, B=B, SBK=SBK, NPAD=NPAD, G=G, GC=GC,
               TA_list=[int(t) for t in TA_list],
               TB_list=[int(t) for t in TB_list],
               offA=[int(o) for o in offA], offB=[int(o) for o in offB],
               TA_tot=TA_tot, TB_tot=TB_tot)
    return cfg, in_maps


# -------------------------------------------------------------- bass program
def _build(cfg):
    import concourse.bacc as bacc
    import concourse.mybir as mybir
    import concourse.tile as tile
    from concourse import bass
    from concourse.library_config import mlp as mlp_lib

    dt = mybir.dt
    B = cfg["B"]
    SBK = cfg["SBK"]
    NPAD = cfg["NPAD"]
    G = cfg["G"]
    GC = cfg["GC"]
    TA_list = cfg["TA_list"]
    TB_list = cfg["TB_list"]
    offA = cfg["offA"]
    offB = cfg["offB"]
    TA_tot = cfg["TA_tot"]
    TB_tot = cfg["TB_tot"]
    T_tot = TA_tot + TB_tot
    rg = [list(range(NC))]

    nc = bacc.Bacc("TRN2", target_bir_lowering=False, num_devices=NC)

    def din(name, shape, dtype):
        return nc.dram_tensor(name, shape, dtype, kind="ExternalInput")

    hT_in = din("hT", [128, NPAD], dt.float32)
    W1_in = din("W1", [128, 128], dt.float32)
    W2_in = din("W2", [128, 128], dt.float32)
    W3_in = din("W3", [128, 1], dt.float32)
    b1_in = din("b1", [128, 1], dt.float32)
    b2_in = din("b2", [128, 1], dt.float32)
    b3_in = din("b3", [128, 1], dt.float32)
    iota4_in = din("iota4", [128, 2048], dt.bfloat16)
    ident_in = din("ident", [128, 128], dt.float32)
    idxA_in = din("idxA16", [128, TA_tot * 8], dt.int16)
    idxB_in = din("idxB16", [128, TB_tot * 8], dt.int16)
    dloc4_in = din("dloc4", [128, T_tot], dt.float32)
    gone_in = din("gonehot", [128, NPAD], dt.bfloat16)
    gidx_in = din("gidx", [128, 1], dt.int32)
    rsl_in = din("rsl", [128, B], dt.int32)
    rsh_in = din("rsh", [128, B], dt.int32)
    rdl_in = din("rdl", [128, B], dt.int32)
    rdh_in = din("rdh", [128, B], dt.int32)
    out_t = nc.dram_tensor("out", [G, 1], dt.float32, kind="ExternalOutput")

    HB2 = NPAD // 2
    x1_loc = nc.dram_tensor("x1_loc", [NPAD, 128], dt.bfloat16)
    x2_loc = nc.dram_tensor("x2_loc", [NPAD, 128], dt.bfloat16)
    x1_fullA = nc.dram_tensor("x1_fullA", [HB2 * NC, 128], dt.bfloat16,
                              addr_space="Shared")
    x1_fullB = nc.dram_tensor("x1_fullB", [HB2 * NC, 128], dt.bfloat16,
                              addr_space="Shared")
    x2_fullA = nc.dram_tensor("x2_fullA", [HB2 * NC, 128], dt.bfloat16,
                              addr_space="Shared")
    x2_fullB = nc.dram_tensor("x2_fullB", [HB2 * NC, 128], dt.bfloat16,
                              addr_space="Shared")
    pool_in = nc.dram_tensor("pool_in", [G + 128, 2], dt.float32)
    pool_out = nc.dram_tensor("pool_out", [G + 128, 2], dt.float32,
                              addr_space="Shared")

    with tile.TileContext(nc) as tc:
        with (
            tc.tile_pool(name="persist", bufs=1) as pp,
            tc.tile_pool(name="work", bufs=3) as wp,
            tc.tile_pool(name="sbuild", bufs=4) as sp_,
            tc.tile_pool(name="gatherA", bufs=4) as gpa,
            tc.tile_pool(name="gatherB", bufs=4) as gpb,
            tc.tile_pool(name="psA", bufs=3, space="PSUM") as psA,
            tc.tile_pool(name="psB", bufs=2, space="PSUM") as psB,
            tc.tile_pool(name="psC", bufs=1, space="PSUM") as psC,
            tc.tile_pool(name="psT", bufs=1, space="PSUM") as psT,
        ):
            # mlp ucode library (dma_gather); must precede all gpsimd work
            nc.gpsimd.load_library(mlp_lib)

            # ---------- constants / weights ----------
            ident = pp.tile([128, 128], dt.float32, tag="ident")
            nc.sync.dma_start(out=ident[:], in_=ident_in[:])
            iota4 = pp.tile([128, 2048], dt.bfloat16, tag="iota4")
            nc.sync.dma_start(out=iota4[:], in_=iota4_in[:])

            def load_w_bf(src_):
                f = wp.tile([128, 128], dt.float32, tag="wload")
                nc.sync.dma_start(out=f[:], in_=src_[:])
                bf = pp.tile([128, 128], dt.bfloat16, tag=src_.name + "bf")
                nc.vector.tensor_copy(out=bf[:], in_=f[:])
                return bf

            W1 = load_w_bf(W1_in)
            W2 = load_w_bf(W2_in)
            W3f = wp.tile([128, 1], dt.float32, tag="w3f")
            nc.sync.dma_start(out=W3f[:], in_=W3_in[:])
            W3 = pp.tile([128, 1], dt.bfloat16, tag="w3bf")
            nc.vector.tensor_copy(out=W3[:], in_=W3f[:])
            b1 = pp.tile([128, 1], dt.float32, tag="b1")
            nc.sync.dma_start(out=b1[:], in_=b1_in[:])
            b2 = pp.tile([128, 1], dt.float32, tag="b2")
            nc.sync.dma_start(out=b2[:], in_=b2_in[:])
            b3 = pp.tile([128, 1], dt.float32, tag="b3")
            nc.sync.dma_start(out=b3[:], in_=b3_in[:])

            # ---------- degree norms ----------
            def make_norm(lo_in, hi_in, tag):
                lo_i = wp.tile([128, B], dt.int32, tag=tag + "loi")
                hi_i = wp.tile([128, B], dt.int32, tag=tag + "hii")
                nc.sync.dma_start(out=lo_i[:], in_=lo_in[:])
                nc.sync.dma_start(out=hi_i[:], in_=hi_in[:])
                lo_f = wp.tile([128, B], dt.float32, tag=tag + "lof")
                hi_f = wp.tile([128, B], dt.float32, tag=tag + "hif")
                nc.vector.tensor_copy(out=lo_f[:], in_=lo_i[:])
                nc.vector.tensor_copy(out=hi_f[:], in_=hi_i[:])
                deg = wp.tile([128, B], dt.float32, tag=tag + "deg")
                nc.vector.tensor_tensor(out=deg[:], in0=hi_f[:], in1=lo_f[:],
                                        op=mybir.AluOpType.subtract)
                nc.vector.tensor_scalar_max(out=deg[:], in0=deg[:], scalar1=1.0)
                rec = wp.tile([128, B], dt.float32, tag=tag + "rec")
                nc.vector.reciprocal(out=rec[:], in_=deg[:])
                nrm = pp.tile([128, B], dt.float32, tag=tag + "nrm")
                nc.scalar.sqrt(out=nrm[:], in_=rec[:])
                return nrm

            norm_src = make_norm(rsl_in, rsh_in, "ns")

            # ---------- edge / pooling metadata ----------
            idxA = pp.tile([128, TA_tot * 8], dt.int16, tag="idxA")
            nc.scalar.dma_start(out=idxA[:], in_=idxA_in[:])
            idxB = pp.tile([128, TB_tot * 8], dt.int16, tag="idxB")
            nc.scalar.dma_start(out=idxB[:], in_=idxB_in[:])
            dloc4 = pp.tile([128, T_tot], dt.float32, tag="dloc4")
            nc.scalar.dma_start(out=dloc4[:], in_=dloc4_in[:])
            gone = pp.tile([128, NPAD], dt.bfloat16, tag="gone")
            nc.scalar.dma_start(out=gone[:], in_=gone_in[:])
            gidx = pp.tile([128, 1], dt.int32, tag="gidx")
            nc.sync.dma_start(out=gidx[:], in_=gidx_in[:])

            # zero pool_in scratch early (only needed before the AllReduce)
            zt = wp.tile([128, 2 * (GC + 1)], dt.float32, tag="zt")
            nc.vector.memset(zt[:], 0.0)
            nc.sync.dma_start(
                out=bass.AP(pool_in, 0, [[2, 128], [256, GC + 1], [1, 2]]),
                in_=zt[:].rearrange("p (g c) -> p g c", c=2))

            # ---------- phase 1: project layer-1 for owned nodes ----------
            CHUNK = 8
            for c0 in range(0, B, CHUNK):
                nb = min(CHUNK, B - c0)
                hf = wp.tile([128, CHUNK * 128], dt.float32, tag="hf")
                nc.sync.dma_start(out=hf[:, :nb * 128],
                                  in_=hT_in[:, c0 * 128:(c0 + nb) * 128])
                hb = wp.tile([128, CHUNK * 128], dt.bfloat16, tag="hb")
                nc.vector.tensor_copy(out=hb[:, :nb * 128], in_=hf[:, :nb * 128])
                for i in range(nb):
                    b = c0 + i
                    ps = psA.tile([128, 128], dt.float32, tag="proj")
                    nc.tensor.matmul(out=ps[:], lhsT=hb[:, i * 128:(i + 1) * 128],
                                     rhs=W1[:], start=True, stop=True)
                    xsb = wp.tile([128, 128], dt.bfloat16, tag="xsb")
                    nc.vector.tensor_copy(out=xsb[:], in_=ps[:])
                    nc.scalar.dma_start(out=x1_loc[b * 128:(b + 1) * 128, :],
                                        in_=xsb[:])

            # ---------- all-gather x1 (two halves) ----------
            nc.gpsimd.collective_compute(
                "AllGather", mybir.AluOpType.bypass, replica_groups=rg,
                ins=[x1_loc[0:HB2, :].opt()], outs=[x1_fullA[:].opt()])
            nc.gpsimd.collective_compute(
                "AllGather", mybir.AluOpType.bypass, replica_groups=rg,
                ins=[x1_loc[HB2:NPAD, :].opt()], outs=[x1_fullB[:].opt()])

            # norm_dst broadcast built here so it hides under the gathers
            norm_dst = make_norm(rdl_in, rdh_in, "nd")
            nd_bc = pp.tile([128, NPAD], dt.float32, tag="ndbc")
            for b in range(B):
                tp = psT.tile([128, 128], dt.float32, tag="ndtp")
                nc.tensor.transpose(
                    out=tp[:],
                    in_=norm_dst[:, b:b + 1].to_broadcast([128, 128]),
                    identity=ident[:],
                )
                nc.vector.tensor_copy(out=nd_bc[:, b * 128:(b + 1) * 128],
                                      in_=tp[:])

            # ---------- aggregation machinery ----------
            def agg_layer(x_fullA, x_fullB, consume_sblock):
                """Aggregate all blocks from pair views of the half tables;
                call consume_sblock(sb, agg) per block."""
                xpA = x_fullA[:].rearrange("(u two) f -> u (two f)", two=2)
                xpB = x_fullB[:].rearrange("(u two) f -> u (two f)", two=2)
                bufsA, bufsB = {}, {}
                gA = [0]
                gB = [0]

                def issue(range_id):
                    if range_id == 0:
                        g, pool, idx_t, tot = gA[0], gpa, idxA, TA_tot
                        bufs = bufsA
                    else:
                        g, pool, idx_t, tot = gB[0], gpb, idxB, TB_tot
                        bufs = bufsB
                    nt = min(GT, tot - g * GT)
                    ni = nt * 128
                    buf = pool.tile([128, GT * 256], dt.bfloat16, tag="g")
                    src_ap = xpA if range_id == 0 else xpB
                    nc.gpsimd.dma_gather(
                        buf[:, :nt * 256].rearrange("p (t e) -> p t e", e=256),
                        src_ap,
                        idx_t[:, g * (GT * 8):g * (GT * 8) + ni // 16],
                        ni, ni, 256,
                    )
                    bufs[g] = buf
                    if range_id == 0:
                        gA[0] += 1
                    else:
                        gB[0] += 1

                for sb in range(SBK):
                    needA = offA[sb] + TA_list[sb]
                    needB = offB[sb] + TB_list[sb]
                    while gA[0] * GT < needA:
                        issue(0)
                    while gB[0] * GT < needB:
                        issue(1)
                    agg = psB.tile([128, 128], dt.float32, tag="agg")
                    n_mm = 2 * (TA_list[sb] + TB_list[sb])
                    i_mm = 0
                    for r, (off_r, T_r, bufs, base_col) in enumerate(
                            ((offA, TA_list, bufsA, 0),
                             (offB, TB_list, bufsB, TA_tot))):
                        j0 = off_r[sb]
                        jend = j0 + T_r[sb]
                        for c0 in range(j0, jend, 8):
                            k = min(8, jend - c0)
                            col = base_col + c0
                            # fused one-hot build for k tiles in one DVE op
                            S4 = sp_.tile([128, 8 * 256], dt.bfloat16,
                                          tag="S4")
                            nc.vector.tensor_tensor(
                                out=S4[:, :k * 256].rearrange(
                                    "p (k w) -> p k w", w=256),
                                in0=iota4[:, :k * 256].rearrange(
                                    "p (k w) -> p k w", w=256),
                                in1=dloc4[:, col:col + k].rearrange(
                                    "p (k one) -> p k one",
                                    one=1).to_broadcast([128, k, 256]),
                                op=mybir.AluOpType.is_equal)
                            for j in range(c0, c0 + k):
                                g, o = divmod(j, GT)
                                buf = bufs[g]
                                s0 = (j - c0) * 256
                                nc.tensor.matmul(
                                    out=agg[:],
                                    lhsT=buf[:, o * 256:o * 256 + 128],
                                    rhs=S4[:, s0:s0 + 128],
                                    start=(i_mm == 0), stop=False)
                                i_mm += 1
                                nc.tensor.matmul(
                                    out=agg[:],
                                    lhsT=buf[:, o * 256 + 128:(o + 1) * 256],
                                    rhs=S4[:, s0 + 128:s0 + 256], start=False,
                                    stop=(i_mm == n_mm - 1))
                                i_mm += 1
                    consume_sblock(sb, agg)

            def finish_h(sb, agg, bias, out_ap):
                t1 = wp.tile([128, 128], dt.float32, tag="t1")
                nc.vector.tensor_tensor(out=t1[:], in0=agg[:],
                                        in1=nd_bc[:, sb * 128:(sb + 1) * 128],
                                        op=mybir.AluOpType.mult)
                nc.scalar.activation(out=out_ap, in_=t1[:],
                                     func=mybir.ActivationFunctionType.Relu,
                                     bias=bias[:, 0:1], scale=1.0)

            # ---------- layer 1 aggregate + layer 2 project ----------
            def consume_l1(sb, agg):
                h1 = wp.tile([128, 128], dt.bfloat16, tag="h1")
                finish_h(sb, agg, b1, h1[:])
                ps2 = psA.tile([128, 128], dt.float32, tag="proj")
                nc.tensor.matmul(out=ps2[:], lhsT=h1[:],
                                 rhs=W2[:], start=True, stop=True)
                x2sb = wp.tile([128, 128], dt.bfloat16, tag="xsb")
                nc.vector.tensor_scalar(out=x2sb[:], in0=ps2[:],
                                        scalar1=norm_src[:, sb:sb + 1],
                                        scalar2=None,
                                        op0=mybir.AluOpType.mult)
                nc.sync.dma_start(out=x2_loc[sb * 128:(sb + 1) * 128, :],
                                  in_=x2sb[:])

            agg_layer(x1_fullA, x1_fullB, consume_l1)

            # ---------- all-gather x2 (two halves) ----------
            nc.gpsimd.collective_compute(
                "AllGather", mybir.AluOpType.bypass, replica_groups=rg,
                ins=[x2_loc[0:HB2, :].opt()], outs=[x2_fullA[:].opt()])
            nc.gpsimd.collective_compute(
                "AllGather", mybir.AluOpType.bypass, replica_groups=rg,
                ins=[x2_loc[HB2:NPAD, :].opt()], outs=[x2_fullB[:].opt()])

            # ---------- layer 2 aggregate + pooling ----------
            pool_acc = pp.tile([128, 2], dt.float32, tag="poolacc")
            nc.vector.memset(pool_acc[:], 0.0)
            ones_col = pp.tile([128, 1], dt.bfloat16, tag="ones")
            nc.vector.memset(ones_col[:], 1.0)

            def consume_l2(sb, agg):
                h2 = wp.tile([128, 128], dt.bfloat16, tag="h2")
                finish_h(sb, agg, b2, h2[:])
                psd = psC.tile([128, 1], dt.float32, tag="dots")
                nc.tensor.matmul(out=psd[:], lhsT=h2[:], rhs=W3[:],
                                 start=True, stop=True)
                rhs2 = wp.tile([128, 2], dt.bfloat16, tag="rhs2")
                nc.vector.tensor_copy(out=rhs2[:, 0:1], in_=psd[:])
                nc.vector.tensor_copy(out=rhs2[:, 1:2], in_=ones_col[:])
                psp = psC.tile([128, 2], dt.float32, tag="poolmm")
                nc.tensor.matmul(out=psp[:],
                                 lhsT=gone[:, sb * 128:(sb + 1) * 128],
                                 rhs=rhs2[:], start=True, stop=True)
                nc.vector.tensor_add(out=pool_acc[:], in0=pool_acc[:],
                                     in1=psp[:])

            agg_layer(x2_fullA, x2_fullB, consume_l2)

            # ---------- combine pools across cores ----------
            nc.gpsimd.indirect_dma_start(
                out=pool_in[:],
                out_offset=bass.IndirectOffsetOnAxis(ap=gidx[:, 0:1], axis=0),
                in_=pool_acc[:], in_offset=None)
            nc.gpsimd.collective_compute(
                "AllReduce", mybir.AluOpType.add, replica_groups=rg,
                ins=[pool_in[:].opt()], outs=[pool_out[:].opt()])

            # ---------- finish: out = dot/cnt + b3 ----------
            dots_t = wp.tile([128, GC], dt.float32, tag="dotst")
            cnt = wp.tile([128, GC], dt.float32, tag="cnt")
            nc.sync.dma_start(
                out=dots_t[:], in_=bass.AP(pool_out, 0, [[2, 128], [256, GC]]))
            nc.sync.dma_start(
                out=cnt[:], in_=bass.AP(pool_out, 1, [[2, 128], [256, GC]]))
            nc.vector.tensor_scalar_max(out=cnt[:], in0=cnt[:], scalar1=1.0)
            rec = wp.tile([128, GC], dt.float32, tag="recc")
            nc.vector.reciprocal(out=rec[:], in_=cnt[:])
            res = wp.tile([128, GC], dt.float32, tag="res")
            nc.vector.tensor_tensor(out=res[:], in0=dots_t[:], in1=rec[:],
                                    op=mybir.AluOpType.mult)
            nc.vector.tensor_scalar(out=res[:], in0=res[:], scalar1=b3[:, 0:1],
                                    scalar2=None, op0=mybir.AluOpType.add)
            nc.sync.dma_start(
                out=bass.AP(out_t, 0, [[1, 128], [128, GC]]), in_=res[:])

    nc.compile()
    return nc


_CACHE = {}


def _get_nc(cfg):
    key = (cfg["N"], cfg["G"], tuple(cfg["TA_list"]), tuple(cfg["TB_list"]))
    if key not in _CACHE:
        _CACHE[key] = _build(cfg)
    return _CACHE[key]


def kernel(**inputs) -> np.ndarray:
    from concourse import bass_utils

    cfg, in_maps = _prep(**inputs)
    nc = _get_nc(cfg)
    res = bass_utils.run_bass_kernel_spmd(
        nc, in_maps, core_ids=list(range(NC)), trace=False)
    return np.asarray(res.results[0]["out"], dtype=np.float32)


# revision 27
# speedup vs baseline: 1.0272x; 1.0272x over previous
"""Trainium2 Bass kernel for a 2-layer GCN regressor (gnn_message_passing).

Computation (matches the reference):
  deg_out/deg_in -> norm = max(deg,1)^-0.5
  layer:  x = (h * norm_src) @ W ; agg = segment_sum(x[src], dst) ;
          h' = relu(agg * norm_dst + b)
  pool:   per-graph mean over nodes, then hg @ W3 + b3 -> [G, 1]

Sharding: nodes are partitioned contiguously across the 8 cores (dst
partitioning).  Each core projects its owned nodes, the projected features are
AllGathered so every core holds the full [N,128] bf16 table in HBM, then each
core aggregates its owned destination super-blocks (256 nodes).

The gather of source rows is the bottleneck (SWDGE descriptor generation on
the gpsimd Q7 cluster is ~8.5 ns/row regardless of instruction).  To minimise
per-instruction overhead the kernel uses dma_gather (mlp ucode library) with
1024 rows per instruction.  dma_gather indices are int16, so the table is
viewed as 50176 pair-rows of 512B (two nodes per row) and split into two
ranges (<32768 rows each).  Edges are packed into 128-slot tiles per
(dst-super-block, range); each gathered 512B slot holds the needed node in
its even or odd half.  A single is_equal against a 512-wide fp16 iota builds
a parity-aware one-hot S4 (col = parity*256 + dst_slot), and two matmuls
accumulate agg^T[128f, 256d] = X_e.T @ S4[:, :256] + X_o.T @ S4[:, 256:].

All heavy matmul traffic is bf16 (one-hot values exact); accumulation fp32.
"""

import numpy as np
import ml_dtypes

BF16 = ml_dtypes.bfloat16
NC = 8          # cores
GT = 8          # gather tiles (of 128 rows) per dma_gather instruction
RSPLIT = 32768  # pair-rows per int16-addressable range


# ----------------------------------------------------------------- host prep
def _prep(h, src, dst, graph_ids, num_graphs, W1, b1, W2, b2, W3, b3):
    h = np.asarray(h, dtype=np.float32)
    src = np.asarray(src, dtype=np.int64)
    dst = np.asarray(dst, dtype=np.int64)
    gid = np.asarray(graph_ids, dtype=np.int64)
    G = int(num_graphs)
    N, D = h.shape
    assert D == 128
    assert N % NC == 0
    NPC = N // NC               # real nodes per core
    B = (NPC + 127) // 128      # 128-node blocks per core
    NPAD = B * 128
    assert B % 2 == 0
    SBK = B                     # tile-grid granularity: one 128-node block
    assert G % 128 == 0
    GC = G // 128               # output column groups

    owner = dst // NPC                      # edge -> owning core
    d_loc = dst - owner * NPC               # local node id on owner
    sblk = d_loc // 128
    jsb = d_loc - sblk * 128                # slot within block
    # source half-table id / pair-row within half / parity
    HB2 = 63 * 128                          # nodes in half 0 per core
    HR = NPAD - HB2                         # nodes in half 1 per core
    assert HB2 * NC // 2 < 32768 and HR * NC // 2 < 32768
    s_owner = src // NPC
    s_pos = src - s_owner * NPC
    rng = (s_pos >= HB2).astype(np.int64)   # half 0 / 1
    stride = np.where(rng, HR // 2, HB2 // 2)
    s_u = s_owner * stride + ((s_pos - rng * HB2) >> 1)
    s_par = (s_pos & 1).astype(np.int64)

    # per (core, sblock, range) edge counts -> shared tile grid
    counts = np.zeros((NC, SBK, 2), dtype=np.int64)
    np.add.at(counts, (owner, sblk, rng), 1)
    T_r = np.maximum(1, (counts.max(axis=0) + 127) // 128)  # [SBK, 2]
    TA_list = T_r[:, 0]
    TB_list = T_r[:, 1]
    TA_tot = int(TA_list.sum())
    TB_tot = int(TB_list.sum())
    T_tot = TA_tot + TB_tot
    offA = np.zeros(SBK, dtype=np.int64)
    offA[1:] = np.cumsum(TA_list)[:-1]
    offB = np.zeros(SBK, dtype=np.int64)
    offB[1:] = np.cumsum(TB_list)[:-1]

    # CSR row pointers (degree metadata) over real node ids
    src_sorted = np.sort(src)
    dst_sorted = np.sort(dst)
    rp_src = np.searchsorted(src_sorted, np.arange(N + 1)).astype(np.int32)
    rp_dst = np.searchsorted(dst_sorted, np.arange(N + 1)).astype(np.int32)

    per_core = []
    order_all = np.lexsort((s_u, rng, sblk, owner))
    e_owner = owner[order_all]
    e_sblk = sblk[order_all]
    e_rng = rng[order_all]
    e_j = jsb[order_all]
    e_u = s_u[order_all]
    e_par = s_par[order_all]
    core_starts = np.searchsorted(e_owner, np.arange(NC + 1))

    def wrap16(seq):
        """int16 wrapped layout [128, len/16]: pos i -> [i%16, i//16], x8."""
        n = len(seq)
        assert n % 16 == 0
        w = np.zeros((128, n // 16), dtype=np.int16)
        t = seq.reshape(-1, 16).T.astype(np.int16)
        for g in range(8):
            w[g * 16:(g + 1) * 16, :] = t
        return w

    for k in range(NC):
        lo, hi = core_starts[k], core_starts[k + 1]
        kb = e_sblk[lo:hi]
        kr = e_rng[lo:hi]
        kj = e_j[lo:hi]
        ku = e_u[lo:hi]
        kp = e_par[lo:hi]
        key = kb * 2 + kr
        bkt_starts = np.searchsorted(key, np.arange(2 * SBK + 1))

        idxA = np.zeros(TA_tot * 128, dtype=np.int64)
        idxB = np.zeros(TB_tot * 128, dtype=np.int64)
        dloc4 = np.full((128, T_tot), -1.0, dtype=np.float32)
        for b in range(SBK):
            for r, (idx_arr, off_r, base_col) in enumerate(
                    ((idxA, offA, 0), (idxB, offB, TA_tot))):
                s, e = bkt_starts[b * 2 + r], bkt_starts[b * 2 + r + 1]
                cnt = e - s
                if cnt == 0:
                    continue
                i = np.arange(cnt)
                t0 = off_r[b]
                slot = t0 * 128 + i          # sequence position in range r
                idx_arr[slot] = ku[s:e]
                col = base_col + t0 + i // 128
                row = i % 128
                dloc4[row, col] = (kp[s:e] * 128 + kj[s:e]).astype(np.float32)

        idxA16 = wrap16(idxA)
        idxB16 = wrap16(idxB)

        # graph one-hot over this core's graph window
        n0 = k * NPC
        kgid = gid[n0:n0 + NPC]
        g_base = int(kgid[0])
        span = int(kgid[-1]) - g_base + 1
        assert span <= 128, f"graph span {span} > 128 on core {k}"
        gone = np.zeros((128, NPAD), dtype=BF16)
        p_all = np.arange(NPC)
        gone[p_all % 128, (p_all // 128) * 128 + (kgid - g_base)] = BF16(1.0)
        gidx = (g_base + np.arange(128)).astype(np.int32)[:, None]

        # row-pointer tiles [128, B] (pad slots -> deg 0)
        node = n0 + np.arange(NPAD)
        valid = node < n0 + NPC
        nn = np.where(valid, node, n0)
        rsl = np.where(valid, rp_src[nn], 0).astype(np.int32).reshape(B, 128).T
        rsh = np.where(valid, rp_src[nn + 1], 0).astype(np.int32).reshape(B, 128).T
        rdl = np.where(valid, rp_dst[nn], 0).astype(np.int32).reshape(B, 128).T
        rdh = np.where(valid, rp_dst[nn + 1], 0).astype(np.int32).reshape(B, 128).T

        deg_out = (rp_src[n0 + 1:n0 + NPC + 1] - rp_src[n0:n0 + NPC])
        nsrc = 1.0 / np.sqrt(np.maximum(deg_out, 1.0))
        hT = np.zeros((128, NPAD), dtype=np.float32)
        hT[:, :NPC] = (h[n0:n0 + NPC] * nsrc[:, None]).T

        per_core.append(dict(
            hT=np.ascontiguousarray(hT),
            idxA16=np.ascontiguousarray(idxA16),
            idxB16=np.ascontiguousarray(idxB16),
            dloc4=np.ascontiguousarray(dloc4),
            gonehot=gone, gidx=gidx,
            rsl=np.ascontiguousarray(rsl), rsh=np.ascontiguousarray(rsh),
            rdl=np.ascontiguousarray(rdl), rdh=np.ascontiguousarray(rdh),
        ))

    iota4 = np.broadcast_to(np.tile(np.arange(256, dtype=np.float32), 8),
                            (128, 2048)).astype(BF16)
    ident = np.eye(128, dtype=np.float32)
    shared = dict(
        W1=np.asarray(W1, np.float32), W2=np.asarray(W2, np.float32),
        W3=np.asarray(W3, np.float32).reshape(128, 1),
        b1=np.asarray(b1, np.float32).reshape(128, 1),
        b2=np.asarray(b2, np.float32).reshape(128, 1),
        b3=np.broadcast_to(np.asarray(b3, np.float32).reshape(1, 1),
                           (128, 1)).astype(np.float32),
        iota4=np.ascontiguousarray(iota4),
        ident=np.ascontiguousarray(ident),
    )
    in_maps = [{**shared, **pc} for pc in per_core]
    cfg = dict(N=N, NPC=NPC, B=B, SBK=SBK, NPAD=NPAD, G=G, GC=GC,
               TA_list=[int(t) for t in TA_list],
               TB_list=[int(t) for t in TB_list],
               offA=[int(o) for o in offA], offB=[int(o) for o in offB],
               TA_tot=TA_tot, TB_tot=TB_tot)
    return cfg, in_maps


# -------------------------------------------------------------- bass program
def _build(cfg):
    import concourse.bacc as bacc
    import concourse.mybir as mybir
    import concourse.tile as tile
    from concourse import bass
    from concourse.library_config import mlp as mlp_lib

    dt = mybir.dt
    B = cfg["B"]
    SBK = cfg["SBK"]
    NPAD = cfg["NPAD"]
    G = cfg["G"]
    GC = cfg["GC"]
    TA_list = cfg["TA_list"]
    TB_list = cfg["TB_list"]
    offA = cfg["offA"]
    offB = cfg["offB"]
    TA_tot = cfg["TA_tot"]
    TB_tot = cfg["TB_tot"]
    T_tot = TA_tot + TB_tot
    rg = [list(range(NC))]

    nc = bacc.Bacc("TRN2", target_bir_lowering=False, num_devices=NC)

    def din(name, shape, dtype):
        return nc.dram_tensor(name, shape, dtype, kind="ExternalInput")

    hT_in = din("hT", [128, NPAD], dt.float32)
    W1_in = din("W1", [128, 128], dt.float32)
    W2_in = din("W2", [128, 128], dt.float32)
    W3_in = din("W3", [128, 1], dt.float32)
    b1_in = din("b1", [128, 1], dt.float32)
    b2_in = din("b2", [128, 1], dt.float32)
    b3_in = din("b3", [128, 1], dt.float32)
    iota4_in = din("iota4", [128, 2048], dt.bfloat16)
    ident_in = din("ident", [128, 128], dt.float32)
    idxA_in = din("idxA16", [128, TA_tot * 8], dt.int16)
    idxB_in = din("idxB16", [128, TB_tot * 8], dt.int16)
    dloc4_in = din("dloc4", [128, T_tot], dt.float32)
    gone_in = din("gonehot", [128, NPAD], dt.bfloat16)
    gidx_in = din("gidx", [128, 1], dt.int32)
    rsl_in = din("rsl", [128, B], dt.int32)
    rsh_in = din("rsh", [128, B], dt.int32)
    rdl_in = din("rdl", [128, B], dt.int32)
    rdh_in = din("rdh", [128, B], dt.int32)
    out_t = nc.dram_tensor("out", [G, 1], dt.float32, kind="ExternalOutput")

    HB2 = 63 * 128
    HR = NPAD - HB2
    x1_loc = nc.dram_tensor("x1_loc", [NPAD, 128], dt.bfloat16)
    x2_loc = nc.dram_tensor("x2_loc", [NPAD, 128], dt.bfloat16)
    x1_fullA = nc.dram_tensor("x1_fullA", [HB2 * NC, 128], dt.bfloat16,
                              addr_space="Shared")
    x1_fullB = nc.dram_tensor("x1_fullB", [HR * NC, 128], dt.bfloat16,
                              addr_space="Shared")
    x2_fullA = nc.dram_tensor("x2_fullA", [HB2 * NC, 128], dt.bfloat16,
                              addr_space="Shared")
    x2_fullB = nc.dram_tensor("x2_fullB", [HR * NC, 128], dt.bfloat16,
                              addr_space="Shared")
    pool_in = nc.dram_tensor("pool_in", [G + 128, 2], dt.float32)
    pool_out = nc.dram_tensor("pool_out", [G + 128, 2], dt.float32,
                              addr_space="Shared")

    with tile.TileContext(nc) as tc:
        with (
            tc.tile_pool(name="persist", bufs=1) as pp,
            tc.tile_pool(name="work", bufs=3) as wp,
            tc.tile_pool(name="sbuild", bufs=4) as sp_,
            tc.tile_pool(name="gatherA", bufs=4) as gpa,
            tc.tile_pool(name="gatherB", bufs=4) as gpb,
            tc.tile_pool(name="psA", bufs=3, space="PSUM") as psA,
            tc.tile_pool(name="psB", bufs=2, space="PSUM") as psB,
            tc.tile_pool(name="psC", bufs=1, space="PSUM") as psC,
            tc.tile_pool(name="psT", bufs=1, space="PSUM") as psT,
        ):
            # mlp ucode library (dma_gather); must precede all gpsimd work
            nc.gpsimd.load_library(mlp_lib)

            # ---------- constants / weights ----------
            ident = pp.tile([128, 128], dt.float32, tag="ident")
            nc.sync.dma_start(out=ident[:], in_=ident_in[:])
            iota4 = pp.tile([128, 2048], dt.bfloat16, tag="iota4")
            nc.sync.dma_start(out=iota4[:], in_=iota4_in[:])

            def load_w_bf(src_):
                f = wp.tile([128, 128], dt.float32, tag="wload")
                nc.sync.dma_start(out=f[:], in_=src_[:])
                bf = pp.tile([128, 128], dt.bfloat16, tag=src_.name + "bf")
                nc.vector.tensor_copy(out=bf[:], in_=f[:])
                return bf

            W1 = load_w_bf(W1_in)
            W2 = load_w_bf(W2_in)
            W3f = wp.tile([128, 1], dt.float32, tag="w3f")
            nc.sync.dma_start(out=W3f[:], in_=W3_in[:])
            W3 = pp.tile([128, 1], dt.bfloat16, tag="w3bf")
            nc.vector.tensor_copy(out=W3[:], in_=W3f[:])
            b1 = pp.tile([128, 1], dt.float32, tag="b1")
            nc.sync.dma_start(out=b1[:], in_=b1_in[:])
            b2 = pp.tile([128, 1], dt.float32, tag="b2")
            nc.sync.dma_start(out=b2[:], in_=b2_in[:])
            b3 = pp.tile([128, 1], dt.float32, tag="b3")
            nc.sync.dma_start(out=b3[:], in_=b3_in[:])

            # ---------- degree norms ----------
            def make_norm(lo_in, hi_in, tag):
                lo_i = wp.tile([128, B], dt.int32, tag=tag + "loi")
                hi_i = wp.tile([128, B], dt.int32, tag=tag + "hii")
                nc.sync.dma_start(out=lo_i[:], in_=lo_in[:])
                nc.sync.dma_start(out=hi_i[:], in_=hi_in[:])
                lo_f = wp.tile([128, B], dt.float32, tag=tag + "lof")
                hi_f = wp.tile([128, B], dt.float32, tag=tag + "hif")
                nc.vector.tensor_copy(out=lo_f[:], in_=lo_i[:])
                nc.vector.tensor_copy(out=hi_f[:], in_=hi_i[:])
                deg = wp.tile([128, B], dt.float32, tag=tag + "deg")
                nc.vector.tensor_tensor(out=deg[:], in0=hi_f[:], in1=lo_f[:],
                                        op=mybir.AluOpType.subtract)
                nc.vector.tensor_scalar_max(out=deg[:], in0=deg[:], scalar1=1.0)
                rec = wp.tile([128, B], dt.float32, tag=tag + "rec")
                nc.vector.reciprocal(out=rec[:], in_=deg[:])
                nrm = pp.tile([128, B], dt.float32, tag=tag + "nrm")
                nc.scalar.sqrt(out=nrm[:], in_=rec[:])
                return nrm

            norm_src = make_norm(rsl_in, rsh_in, "ns")

            # ---------- edge / pooling metadata ----------
            idxA = pp.tile([128, TA_tot * 8], dt.int16, tag="idxA")
            nc.scalar.dma_start(out=idxA[:], in_=idxA_in[:])
            idxB = pp.tile([128, TB_tot * 8], dt.int16, tag="idxB")
            nc.scalar.dma_start(out=idxB[:], in_=idxB_in[:])
            dloc4 = pp.tile([128, T_tot], dt.float32, tag="dloc4")
            nc.scalar.dma_start(out=dloc4[:], in_=dloc4_in[:])
            gone = pp.tile([128, NPAD], dt.bfloat16, tag="gone")
            nc.scalar.dma_start(out=gone[:], in_=gone_in[:])
            gidx = pp.tile([128, 1], dt.int32, tag="gidx")
            nc.sync.dma_start(out=gidx[:], in_=gidx_in[:])

            # zero pool_in scratch early (only needed before the AllReduce)
            zt = wp.tile([128, 2 * (GC + 1)], dt.float32, tag="zt")
            nc.vector.memset(zt[:], 0.0)
            nc.sync.dma_start(
                out=bass.AP(pool_in, 0, [[2, 128], [256, GC + 1], [1, 2]]),
                in_=zt[:].rearrange("p (g c) -> p g c", c=2))

            # ---------- phase 1: project layer-1 for owned nodes ----------
            CHUNK = 8
            for c0 in range(0, B, CHUNK):
                nb = min(CHUNK, B - c0)
                hf = wp.tile([128, CHUNK * 128], dt.float32, tag="hf")
                nc.sync.dma_start(out=hf[:, :nb * 128],
                                  in_=hT_in[:, c0 * 128:(c0 + nb) * 128])
                hb = wp.tile([128, CHUNK * 128], dt.bfloat16, tag="hb")
                nc.vector.tensor_copy(out=hb[:, :nb * 128], in_=hf[:, :nb * 128])
                for i in range(nb):
                    b = c0 + i
                    ps = psA.tile([128, 128], dt.float32, tag="proj")
                    nc.tensor.matmul(out=ps[:], lhsT=hb[:, i * 128:(i + 1) * 128],
                                     rhs=W1[:], start=True, stop=True)
                    xsb = wp.tile([128, 128], dt.bfloat16, tag="xsb")
                    nc.vector.tensor_copy(out=xsb[:], in_=ps[:])
                    nc.scalar.dma_start(out=x1_loc[b * 128:(b + 1) * 128, :],
                                        in_=xsb[:])

            # ---------- all-gather x1 (two halves) ----------
            nc.gpsimd.collective_compute(
                "AllGather", mybir.AluOpType.bypass, replica_groups=rg,
                ins=[x1_loc[0:HB2, :].opt()], outs=[x1_fullA[:].opt()])
            nc.gpsimd.collective_compute(
                "AllGather", mybir.AluOpType.bypass, replica_groups=rg,
                ins=[x1_loc[HB2:NPAD, :].opt()], outs=[x1_fullB[:].opt()])

            # norm_dst broadcast built here so it hides under the gathers
            norm_dst = make_norm(rdl_in, rdh_in, "nd")
            nd_bc = pp.tile([128, NPAD], dt.float32, tag="ndbc")
            for b in range(B):
                tp = psT.tile([128, 128], dt.float32, tag="ndtp")
                nc.tensor.transpose(
                    out=tp[:],
                    in_=norm_dst[:, b:b + 1].to_broadcast([128, 128]),
                    identity=ident[:],
                )
                nc.vector.tensor_copy(out=nd_bc[:, b * 128:(b + 1) * 128],
                                      in_=tp[:])

            # ---------- aggregation machinery ----------
            def agg_layer(x_fullA, x_fullB, consume_sblock):
                """Aggregate all blocks from pair views of the half tables;
                call consume_sblock(sb, agg) per block."""
                xpA = x_fullA[:].rearrange("(u two) f -> u (two f)", two=2)
                xpB = x_fullB[:].rearrange("(u two) f -> u (two f)", two=2)
                bufsA, bufsB = {}, {}
                gA = [0]
                gB = [0]

                def issue(range_id):
                    if range_id == 0:
                        g, pool, idx_t, tot = gA[0], gpa, idxA, TA_tot
                        bufs = bufsA
                    else:
                        g, pool, idx_t, tot = gB[0], gpb, idxB, TB_tot
                        bufs = bufsB
                    nt = min(GT, tot - g * GT)
                    ni = nt * 128
                    buf = pool.tile([128, GT * 256], dt.bfloat16, tag="g")
                    src_ap = xpA if range_id == 0 else xpB
                    nc.gpsimd.dma_gather(
                        buf[:, :nt * 256].rearrange("p (t e) -> p t e", e=256),
                        src_ap,
                        idx_t[:, g * (GT * 8):g * (GT * 8) + ni // 16],
                        ni, ni, 256,
                    )
                    bufs[g] = buf
                    if range_id == 0:
                        gA[0] += 1
                    else:
                        gB[0] += 1

                for sb in range(SBK):
                    needA = offA[sb] + TA_list[sb]
                    needB = offB[sb] + TB_list[sb]
                    while gA[0] * GT < needA:
                        issue(0)
                    while gB[0] * GT < needB:
                        issue(1)
                    agg = psB.tile([128, 128], dt.float32, tag="agg")
                    n_mm = 2 * (TA_list[sb] + TB_list[sb])
                    i_mm = 0
                    for r, (off_r, T_r, bufs, base_col) in enumerate(
                            ((offA, TA_list, bufsA, 0),
                             (offB, TB_list, bufsB, TA_tot))):
                        j0 = off_r[sb]
                        jend = j0 + T_r[sb]
                        for c0 in range(j0, jend, 8):
                            k = min(8, jend - c0)
                            col = base_col + c0
                            # fused one-hot build for k tiles in one DVE op
                            S4 = sp_.tile([128, 8 * 256], dt.bfloat16,
                                          tag="S4")
                            nc.vector.tensor_tensor(
                                out=S4[:, :k * 256].rearrange(
                                    "p (k w) -> p k w", w=256),
                                in0=iota4[:, :k * 256].rearrange(
                                    "p (k w) -> p k w", w=256),
                                in1=dloc4[:, col:col + k].rearrange(
                                    "p (k one) -> p k one",
                                    one=1).to_broadcast([128, k, 256]),
                                op=mybir.AluOpType.is_equal)
                            for j in range(c0, c0 + k):
                                g, o = divmod(j, GT)
                                buf = bufs[g]
                                s0 = (j - c0) * 256
                                nc.tensor.matmul(
                                    out=agg[:],
                                    lhsT=buf[:, o * 256:o * 256 + 128],
                                    rhs=S4[:, s0:s0 + 128],
                                    start=(i_mm == 0), stop=False)
                                i_mm += 1
                                nc.tensor.matmul(
                                    out=agg[:],
                                    lhsT=buf[:, o * 256 + 128:(o + 1) * 256],
                                    rhs=S4[:, s0 + 128:s0 + 256], start=False,
                                    stop=(i_mm == n_mm - 1))
                                i_mm += 1
                    consume_sblock(sb, agg)

            def finish_h(sb, agg, bias, out_ap):
                t1 = wp.tile([128, 128], dt.float32, tag="t1")
                nc.vector.tensor_tensor(out=t1[:], in0=agg[:],
                                        in1=nd_bc[:, sb * 128:(sb + 1) * 128],
                                        op=mybir.AluOpType.mult)
                nc.scalar.activation(out=out_ap, in_=t1[:],
                                     func=mybir.ActivationFunctionType.Relu,
                                     bias=bias[:, 0:1], scale=1.0)

            # ---------- layer 1 aggregate + layer 2 project ----------
            def consume_l1(sb, agg):
                h1 = wp.tile([128, 128], dt.bfloat16, tag="h1")
                finish_h(sb, agg, b1, h1[:])
                ps2 = psA.tile([128, 128], dt.float32, tag="proj")
                nc.tensor.matmul(out=ps2[:], lhsT=h1[:],
                                 rhs=W2[:], start=True, stop=True)
                x2sb = wp.tile([128, 128], dt.bfloat16, tag="xsb")
                nc.vector.tensor_scalar(out=x2sb[:], in0=ps2[:],
                                        scalar1=norm_src[:, sb:sb + 1],
                                        scalar2=None,
                                        op0=mybir.AluOpType.mult)
                nc.sync.dma_start(out=x2_loc[sb * 128:(sb + 1) * 128, :],
                                  in_=x2sb[:])

            agg_layer(x1_fullA, x1_fullB, consume_l1)

            # ---------- all-gather x2 (two halves) ----------
            nc.gpsimd.collective_compute(
                "AllGather", mybir.AluOpType.bypass, replica_groups=rg,
                ins=[x2_loc[0:HB2, :].opt()], outs=[x2_fullA[:].opt()])
            nc.gpsimd.collective_compute(
                "AllGather", mybir.AluOpType.bypass, replica_groups=rg,
                ins=[x2_loc[HB2:NPAD, :].opt()], outs=[x2_fullB[:].opt()])

            # ---------- layer 2 aggregate + pooling ----------
            pool_acc = pp.tile([128, 2], dt.float32, tag="poolacc")
            nc.vector.memset(pool_acc[:], 0.0)
            ones_col = pp.tile([128, 1], dt.bfloat16, tag="ones")
            nc.vector.memset(ones_col[:], 1.0)

            def consume_l2(sb, agg):
                h2 = wp.tile([128, 128], dt.bfloat16, tag="h2")
                finish_h(sb, agg, b2, h2[:])
                psd = psC.tile([128, 1], dt.float32, tag="dots")
                nc.tensor.matmul(out=psd[:], lhsT=h2[:], rhs=W3[:],
                                 start=True, stop=True)
                rhs2 = wp.tile([128, 2], dt.bfloat16, tag="rhs2")
                nc.vector.tensor_copy(out=rhs2[:, 0:1], in_=psd[:])
                nc.vector.tensor_copy(out=rhs2[:, 1:2], in_=ones_col[:])
                psp = psC.tile([128, 2], dt.float32, tag="poolmm")
                nc.tensor.matmul(out=psp[:],
                                 lhsT=gone[:, sb * 128:(sb + 1) * 128],
                                 rhs=rhs2[:], start=True, stop=True)
                nc.vector.tensor_add(out=pool_acc[:], in0=pool_acc[:],
                                     in1=psp[:])

            agg_layer(x2_fullA, x2_fullB, consume_l2)

            # ---------- combine pools across cores ----------
            nc.gpsimd.indirect_dma_start(
                out=pool_in[:],
                out_offset=bass.IndirectOffsetOnAxis(ap=gidx[:, 0:1], axis=0),
                in_=pool_acc[:], in_offset=None)
            nc.gpsimd.collective_compute(
                "AllReduce", mybir.AluOpType.add, replica_groups=rg,
                ins=[pool_in[:].opt()], outs=[pool_out[:].opt()])

            # ---------- finish: out = dot/cnt + b3 ----------
            dots_t = wp.tile([128, GC], dt.float32, tag="dotst")
            cnt = wp.tile([128, GC], dt.float32, tag="cnt")
            nc.sync.dma_start(
                out=dots_t[:], in_=bass.AP(pool_out, 0, [[2, 128], [256, GC]]))
            nc.sync.dma_start(
                out=cnt[:], in_=bass.AP(pool_out, 1, [[2, 128], [256, GC]]))
            nc.vector.tensor_scalar_max(out=cnt[:], in0=cnt[:], scalar1=1.0)
            rec = wp.tile([128, GC], dt.float32, tag="recc")
            nc.vector.reciprocal(out=rec[:], in_=cnt[:])
            res = wp.tile([128, GC], dt.float32, tag="res")
            nc.vector.tensor_tensor(out=res[:], in0=dots_t[:], in1=rec[:],
                                    op=mybir.AluOpType.mult)
            nc.vector.tensor_scalar(out=res[:], in0=res[:], scalar1=b3[:, 0:1],
                                    scalar2=None, op0=mybir.AluOpType.add)
            nc.sync.dma_start(
                out=bass.AP(out_t, 0, [[1, 128], [128, GC]]), in_=res[:])

    nc.compile()
    return nc


_CACHE = {}


def _get_nc(cfg):
    key = (cfg["N"], cfg["G"], tuple(cfg["TA_list"]), tuple(cfg["TB_list"]))
    if key not in _CACHE:
        _CACHE[key] = _build(cfg)
    return _CACHE[key]


def kernel(**inputs) -> np.ndarray:
    from concourse import bass_utils

    cfg, in_maps = _prep(**inputs)
    nc = _get_nc(cfg)
    res = bass_utils.run_bass_kernel_spmd(
        nc, in_maps, core_ids=list(range(NC)), trace=False)
    return np.asarray(res.results[0]["out"], dtype=np.float32)
